# revision 21
# baseline (speedup 1.0000x reference)
"""Bahdanau attention Trainium2 kernel.

Problem: B=32, S=2048, H=1024 fp32.
  W1e = einsum('bsh,oh->bso', enc, W1)         # big matmul, 137 GFLOP
  W2h = einsum('bh,oh->bo', hidden, W2)
  score = einsum('bso,o->bs', tanh(W1e + W2h[:,None,:]), v)
  score = where(mask, score, -1e9); attn = softmax(score, -1)
  context = einsum('bs,bsh->bh', attn, enc)
Returns (context, attn).

Sharding: data-parallel over batch, 4 batches per core on 8 cores.

Per-core device layout (everything o- or h-on-partitions):
  pre^T tiles [o=128, r=512]: lhsT = W1^T tiles (host-pretransposed),
  rhs = enc^T tiles streamed directly from a host-pretransposed copy of
  enc.  tanh fused with the +W2h bias on ACT (bias is per-partition in
  this layout).  score via PE matmul with v as the stationary [128,1]
  operand.  softmax on one partition (tiny).  context via PE matmul with
  attn columns as stationary [128,1] and naturally-laid enc as rhs.

Matmul dtype: float32r (fp32 bits, single-pass PE mode, 1 cyc/row at
N>=256) for the large matmuls; exact fp32 elsewhere.
"""

import sys
from contextlib import ExitStack

import numpy as np

for _p in ("/opt/trn_rl_repo",):
    if _p not in sys.path:
        sys.path.insert(0, _p)

import concourse.bass as bass
import concourse.tile as tile
from concourse.masks import make_identity
from concourse import bacc, mybir

B, S, H = 32, 2048, 1024
NCORES = 8
BPC = B // NCORES          # batches per core
HT = H // 128              # h tiles (contraction)
OT = H // 128              # o tiles (output channels of W1/W2)
RCHUNK = 512               # rows (s positions) per chunk
NJ = RCHUNK // 128         # 128-row subtiles per chunk
F32 = mybir.dt.float32
F32R = mybir.dt.float32r
U8 = mybir.dt.uint8


def build_program(bpc=BPC, mm_f32r=True, enable_asserts=False):
    """Emit the per-core Tile program (identical on all cores)."""
    nc = bacc.Bacc(
        "TRN2",
        target_bir_lowering=False,
        debug=False,
        enable_asserts=enable_asserts,
        num_devices=NCORES,
    )
    mm_dt = F32R if mm_f32r else F32

    enc = nc.dram_tensor("enc", [bpc, S, H], F32, kind="ExternalInput").ap()
    encTd = nc.dram_tensor("encT", [bpc, H, S], F32, kind="ExternalInput").ap()
    hidT = nc.dram_tensor("hidT", [H, bpc], F32, kind="ExternalInput").ap()
    madd = nc.dram_tensor("madd", [bpc, S], F32, kind="ExternalInput").ap()
    w1t = nc.dram_tensor("w1t", [H, H], F32, kind="ExternalInput").ap()
    w2t = nc.dram_tensor("w2t", [H, H], F32, kind="ExternalInput").ap()
    vr = nc.dram_tensor("vr", [128, OT], F32, kind="ExternalInput").ap()
    ctx_out = nc.dram_tensor("ctx_out", [bpc, H], F32, kind="ExternalOutput").ap()
    attn_out = nc.dram_tensor("attn_out", [bpc, S], F32, kind="ExternalOutput").ap()

    RC = S // RCHUNK  # r chunks per batch

    with tile.TileContext(nc) as tc, ExitStack() as ctx:
        singles = ctx.enter_context(tc.tile_pool(name="singles", bufs=1))
        w2sp = ctx.enter_context(tc.tile_pool(name="w2sp", bufs=3))
        encTp = ctx.enter_context(tc.tile_pool(name="encTp", bufs=2))
        ectxp = ctx.enter_context(tc.tile_pool(name="ectxp", bufs=4))
        ttp = ctx.enter_context(tc.tile_pool(name="ttp", bufs=2))
        scorep = ctx.enter_context(tc.tile_pool(name="scorep", bufs=1))
        smallp = ctx.enter_context(tc.tile_pool(name="smallp", bufs=2))
        sm1p = ctx.enter_context(tc.tile_pool(name="sm1p", bufs=1))
        ctxp = ctx.enter_context(tc.tile_pool(name="ctxp", bufs=2))

        ppp = ctx.enter_context(tc.tile_pool(name="ppp", bufs=4, space="PSUM"))
        ppv = ctx.enter_context(tc.tile_pool(name="ppv", bufs=2, space="PSUM"))
        ppw = ctx.enter_context(tc.tile_pool(name="ppw", bufs=2, space="PSUM"))

        # ---- resident constants
        w1t_sb = singles.tile([128, HT, H], mm_dt)      # [hp, ht, o]
        hid_sb = singles.tile([128, HT, bpc], F32)      # [hp, ht, b]
        vr_sb = singles.tile([128, OT], mm_dt)          # [op, ot]
        identb = singles.tile([bpc, bpc], F32)
        w2h_bo = singles.tile([bpc, H], F32)            # [b, o]
        w2h_sb = singles.tile([128, OT, bpc], F32)      # [op, ot, b]

        nc.sync.dma_start(out=hid_sb, in_=hidT.rearrange("(t p) b -> p t b", p=128))
        nc.sync.dma_start(out=vr_sb, in_=vr.bitcast(vr_sb.dtype))
        nc.sync.dma_start(
            out=w1t_sb[:, 0, :], in_=w1t[0:128, :].bitcast(w1t_sb.dtype))
        make_identity(nc, identb)

        # ---- W2h = hidden @ W2^T -> [b, o], then transpose to [op, ot, b]
        # (emitted after the first main-matmul group so the PE starts on the
        # big matmuls as soon as the first encT chunk lands)
        def emit_w2h():
            for half in range(2):
                pwh = ppw.tile([bpc, 512], F32, tag="cxs")
                for ht in range(HT):
                    w2ts = w2sp.tile([128, 512], F32)
                    nc.sync.dma_start(
                        out=w2ts,
                        in_=w2t[ht * 128:(ht + 1) * 128,
                                half * 512:(half + 1) * 512])
                    nc.tensor.matmul(
                        pwh,
                        lhsT=hid_sb[:, ht, :],
                        rhs=w2ts,
                        start=(ht == 0),
                        stop=(ht == HT - 1),
                    )
                nc.vector.tensor_copy(w2h_bo[:, half * 512:(half + 1) * 512], pwh)
            for ot in range(OT):
                pwt = ppw.tile([128, bpc], F32, tag="cxs")
                nc.tensor.transpose(
                    pwt, w2h_bo[:, ot * 128:(ot + 1) * 128], identb)
                nc.vector.tensor_copy(w2h_sb[:, ot, :], pwt)

        def emit_softmax(b, score_sb):
            # ---- mask (additive) + softmax, no max-shift: |score| <= ||v||_1
            # <= 32, so exp cannot overflow; exp(score - 1e9) == 0 exactly,
            # matching where(mask, score, -1e9) through the softmax.
            mk = smallp.tile([1, S], F32)
            nc.sync.dma_start(out=mk, in_=madd[b:b + 1, :])
            scm = sm1p.tile([1, S], F32)
            nc.vector.tensor_tensor(
                out=scm, in0=score_sb, in1=mk, op=mybir.AluOpType.add)
            ex = sm1p.tile([1, S], F32)
            ssum = smallp.tile([1, 1], F32)
            nc.scalar.activation(
                ex, scm, mybir.ActivationFunctionType.Exp,
                bias=0.0, scale=1.0, accum_out=ssum,
            )
            rinv = smallp.tile([1, 1], F32)
            nc.vector.reciprocal(rinv, ssum)
            nc.vector.tensor_scalar_mul(ex, ex, rinv)
            nc.sync.dma_start(out=attn_out[b:b + 1, :], in_=ex)
            return ex

        NHC = 4
        HC = H // NHC
        acols = {}

        def emit_ctx_hc(b, hc, ex):
            # ---- context[b, h-chunk] = attn[b, :] @ enc[b][:, h-chunk] on PE
            # (attn columns as stationary; acol[p, j] = attn[b, p*16+j])
            if hc == 0:
                acols[b] = smallp.tile([128, S // 128], mm_dt, tag="acol",
                                       name=f"acol{b}")
                nc.sync.dma_start(out=acols[b], in_=ex.bitcast(mm_dt))
            acol = acols[b]
            h0 = hc * HC
            ectx = ectxp.tile([128, S // 128, HC], mm_dt)
            nc.sync.dma_start(
                out=ectx,
                in_=enc[b].rearrange("(p j) h -> p j h", p=128)
                [:, :, h0:h0 + HC].bitcast(mm_dt),
            )
            pcx = ppw.tile([1, HC], F32, tag="cxs")
            for j in range(S // 128):
                nc.tensor.matmul(
                    pcx,
                    lhsT=acol[:, j:j + 1],
                    rhs=ectx[:, j, :],
                    start=(j == 0),
                    stop=(j == S // 128 - 1),
                )
            cres = ctxp.tile([1, HC], F32)
            nc.vector.tensor_copy(cres, pcx)
            nc.sync.dma_start(out=ctx_out[b:b + 1, h0:h0 + HC], in_=cres)

        encT_tiles = {}

        def load_encT(b, rc, interleave_w1t=False):
            s0 = rc * RCHUNK
            # encT[hp, ht, r] = enc[b, s0+r, ht*128+hp], streamed from the
            # host-pretransposed copy (HWDGE; bitcast relabels f32 -> f32r)
            t = encTp.tile([128, HT, RCHUNK], mm_dt, tag="encT",
                           name=f"encT{b}_{rc}")
            for ht in range(HT):
                nc.sync.dma_start(
                    out=t[:, ht, :],
                    in_=encTd[b, ht * 128:(ht + 1) * 128,
                              s0:s0 + RCHUNK].bitcast(t.dtype),
                )
                if interleave_w1t and ht >= 1:
                    nc.sync.dma_start(
                        out=w1t_sb[:, ht, :],
                        in_=w1t[ht * 128:(ht + 1) * 128, :].bitcast(
                            w1t_sb.dtype))
            encT_tiles[(b, rc)] = t

        load_encT(0, 0, interleave_w1t=True)
        exs = []
        for b in range(bpc):
            score_sb = scorep.tile([1, S], F32)
            for rc in range(RC):
                s0 = rc * RCHUNK
                nb, nrc = (b, rc + 1) if rc + 1 < RC else (b + 1, 0)
                if nb < bpc:
                    load_encT(nb, nrc)
                encT = encT_tiles.pop((b, rc))
                # pre^T = W1^T.T @ encT ; tanh(+W2h) ; score += v^T @ tanh
                pv = ppv.tile([1, RCHUNK], F32)
                for ot in range(OT):
                    pp = ppp.tile([128, RCHUNK], F32)
                    for ht in range(HT):
                        nc.tensor.matmul(
                            pp,
                            lhsT=w1t_sb[:, ht, ot * 128:(ot + 1) * 128],
                            rhs=encT[:, ht, :],
                            start=(ht == 0),
                            stop=(ht == HT - 1),
                        )
                    if b == 0 and rc == 0 and ot == 0:
                        emit_w2h()
                    tt = ttp.tile([128, RCHUNK], mm_dt)
                    nc.scalar.activation(
                        tt, pp, mybir.ActivationFunctionType.Tanh,
                        bias=w2h_sb[:, ot, b:b + 1], scale=1.0,
                    )
                    nc.tensor.matmul(
                        pv,
                        lhsT=vr_sb[:, ot:ot + 1],
                        rhs=tt,
                        start=(ot == 0),
                        stop=(ot == OT - 1),
                    )
                nc.vector.tensor_copy(score_sb[:, s0:s0 + RCHUNK], pv)
                if b > 0:
                    emit_ctx_hc(b - 1, rc, exs[b - 1])
            exs.append(emit_softmax(b, score_sb))
        for hc in range(NHC):
            emit_ctx_hc(bpc - 1, hc, exs[bpc - 1])

    nc.compile()
    return nc


_NC_CACHE = {}


def _get_nc(**kw):
    key = tuple(sorted(kw.items()))
    if key not in _NC_CACHE:
        _NC_CACHE[key] = build_program(**kw)
    return _NC_CACHE[key]


def make_in_maps(hidden_top, encoder_outputs, mask, W1, W2, v, bpc=BPC):
    """Host-side sharding + layout transforms (no model math)."""
    hidden_top = np.ascontiguousarray(hidden_top, dtype=np.float32)
    encoder_outputs = np.ascontiguousarray(encoder_outputs, dtype=np.float32)
    w1tf = np.ascontiguousarray(np.asarray(W1, dtype=np.float32).T)
    w2tf = np.ascontiguousarray(np.asarray(W2, dtype=np.float32).T)
    vrf = np.ascontiguousarray(np.asarray(v, dtype=np.float32).reshape(OT, 128).T)
    madd_f = np.where(np.asarray(mask), np.float32(0), np.float32(-1e9))
    n_cores = hidden_top.shape[0] // bpc
    in_maps = []
    encT_all = np.ascontiguousarray(encoder_outputs.transpose(0, 2, 1))
    for c in range(n_cores):
        sl = slice(c * bpc, (c + 1) * bpc)
        in_maps.append({
            "enc": encoder_outputs[sl],
            "encT": encT_all[sl],
            "hidT": np.ascontiguousarray(hidden_top[sl].T),
            "madd": madd_f[sl],
            "w1t": w1tf,
            "w2t": w2tf,
            "vr": vrf,
        })
    return in_maps


def kernel(hidden_top, encoder_outputs, mask, W1, W2, v):
    from concourse.bass_utils import run_bass_kernel_spmd

    nc = _get_nc()
    in_maps = make_in_maps(hidden_top, encoder_outputs, mask, W1, W2, v)
    res = run_bass_kernel_spmd(nc, in_maps, core_ids=list(range(NCORES)))
    context = np.concatenate([r["ctx_out"] for r in res.results], axis=0)
    attn = np.concatenate([r["attn_out"] for r in res.results], axis=0)
    return (context, attn)


# revision 22
# speedup vs baseline: 1.0327x; 1.0327x over previous
"""Bahdanau attention Trainium2 kernel.

Problem: B=32, S=2048, H=1024 fp32.
  W1e = einsum('bsh,oh->bso', enc, W1)         # big matmul, 137 GFLOP
  W2h = einsum('bh,oh->bo', hidden, W2)
  score = einsum('bso,o->bs', tanh(W1e + W2h[:,None,:]), v)
  score = where(mask, score, -1e9); attn = softmax(score, -1)
  context = einsum('bs,bsh->bh', attn, enc)
Returns (context, attn).

Sharding: data-parallel over batch, 4 batches per core on 8 cores.

Per-core device layout (everything o- or h-on-partitions):
  pre^T tiles [o=128, r=512]: lhsT = W1^T tiles (host-pretransposed),
  rhs = enc^T tiles streamed directly from a host-pretransposed copy of
  enc.  tanh fused with the +W2h bias on ACT (bias is per-partition in
  this layout).  score via PE matmul with v as the stationary [128,1]
  operand.  softmax on one partition (tiny).  context via PE matmul with
  attn columns as stationary [128,1] and naturally-laid enc as rhs.

Matmul dtype: float32r (fp32 bits, single-pass PE mode, 1 cyc/row at
N>=256) for the large matmuls; exact fp32 elsewhere.
"""

import sys
from contextlib import ExitStack

import numpy as np

for _p in ("/opt/trn_rl_repo",):
    if _p not in sys.path:
        sys.path.insert(0, _p)

import concourse.bass as bass
import concourse.tile as tile
from concourse.masks import make_identity
from concourse import bacc, mybir

B, S, H = 32, 2048, 1024
NCORES = 8
BPC = B // NCORES          # batches per core
HT = H // 128              # h tiles (contraction)
OT = H // 128              # o tiles (output channels of W1/W2)
RCHUNK = 512               # rows (s positions) per chunk
NJ = RCHUNK // 128         # 128-row subtiles per chunk
F32 = mybir.dt.float32
F32R = mybir.dt.float32r
U8 = mybir.dt.uint8


def build_program(bpc=BPC, mm_f32r=True, enable_asserts=False):
    """Emit the per-core Tile program (identical on all cores)."""
    nc = bacc.Bacc(
        "TRN2",
        target_bir_lowering=False,
        debug=False,
        enable_asserts=enable_asserts,
        num_devices=NCORES,
    )
    mm_dt = F32R if mm_f32r else F32

    enc = nc.dram_tensor("enc", [bpc, S, H], F32, kind="ExternalInput").ap()
    encTd = nc.dram_tensor("encT", [bpc, H, S], F32, kind="ExternalInput").ap()
    hidT = nc.dram_tensor("hidT", [H, bpc], F32, kind="ExternalInput").ap()
    madd = nc.dram_tensor("madd", [bpc, S], F32, kind="ExternalInput").ap()
    w1t = nc.dram_tensor("w1t", [H, H], F32, kind="ExternalInput").ap()
    w2t = nc.dram_tensor("w2t", [H, H], F32, kind="ExternalInput").ap()
    vr = nc.dram_tensor("vr", [128, OT], F32, kind="ExternalInput").ap()
    ctx_out = nc.dram_tensor("ctx_out", [bpc, H], F32, kind="ExternalOutput").ap()
    attn_out = nc.dram_tensor("attn_out", [bpc, S], F32, kind="ExternalOutput").ap()

    RC = S // RCHUNK  # r chunks per batch

    with tile.TileContext(nc) as tc, ExitStack() as ctx:
        singles = ctx.enter_context(tc.tile_pool(name="singles", bufs=1))
        w2sp = ctx.enter_context(tc.tile_pool(name="w2sp", bufs=3))
        encTp = ctx.enter_context(tc.tile_pool(name="encTp", bufs=2))
        ectxp = ctx.enter_context(tc.tile_pool(name="ectxp", bufs=4))
        ttp = ctx.enter_context(tc.tile_pool(name="ttp", bufs=2))
        scorep = ctx.enter_context(tc.tile_pool(name="scorep", bufs=1))
        smallp = ctx.enter_context(tc.tile_pool(name="smallp", bufs=2))
        sm1p = ctx.enter_context(tc.tile_pool(name="sm1p", bufs=1))
        ctxp = ctx.enter_context(tc.tile_pool(name="ctxp", bufs=2))

        ppp = ctx.enter_context(tc.tile_pool(name="ppp", bufs=4, space="PSUM"))
        ppv = ctx.enter_context(tc.tile_pool(name="ppv", bufs=2, space="PSUM"))
        ppw = ctx.enter_context(tc.tile_pool(name="ppw", bufs=2, space="PSUM"))

        # ---- resident constants
        w1t_sb = singles.tile([128, HT, H], mm_dt)      # [hp, ht, o]
        hid_sb = singles.tile([128, HT, bpc], F32)      # [hp, ht, b]
        vr_sb = singles.tile([128, OT], mm_dt)          # [op, ot]
        identb = singles.tile([bpc, bpc], F32)
        w2h_bo = singles.tile([bpc, H], F32)            # [b, o]
        w2h_sb = singles.tile([128, OT, bpc], F32)      # [op, ot, b]

        nc.sync.dma_start(out=hid_sb, in_=hidT.rearrange("(t p) b -> p t b", p=128))
        nc.sync.dma_start(out=vr_sb, in_=vr.bitcast(vr_sb.dtype))
        nc.sync.dma_start(
            out=w1t_sb[:, 0, :], in_=w1t[0:128, :].bitcast(w1t_sb.dtype))
        make_identity(nc, identb)

        # ---- W2h = hidden @ W2^T -> [b, o], then transpose to [op, ot, b]
        # (emitted after the first main-matmul group so the PE starts on the
        # big matmuls as soon as the first encT chunk lands)
        def emit_w2h():
            for half in range(2):
                pwh = ppw.tile([bpc, 512], F32, tag="cxs")
                for ht in range(HT):
                    w2ts = w2sp.tile([128, 512], F32)
                    nc.sync.dma_start(
                        out=w2ts,
                        in_=w2t[ht * 128:(ht + 1) * 128,
                                half * 512:(half + 1) * 512])
                    nc.tensor.matmul(
                        pwh,
                        lhsT=hid_sb[:, ht, :],
                        rhs=w2ts,
                        start=(ht == 0),
                        stop=(ht == HT - 1),
                    )
                nc.vector.tensor_copy(w2h_bo[:, half * 512:(half + 1) * 512], pwh)
            for ot in range(OT):
                pwt = ppw.tile([128, bpc], F32, tag="cxs")
                nc.tensor.transpose(
                    pwt, w2h_bo[:, ot * 128:(ot + 1) * 128], identb)
                nc.vector.tensor_copy(w2h_sb[:, ot, :], pwt)

        def emit_softmax(b, score_sb):
            # ---- mask (additive) + softmax, no max-shift: |score| <= ||v||_1
            # <= 32, so exp cannot overflow; exp(score - 1e9) == 0 exactly,
            # matching where(mask, score, -1e9) through the softmax.
            mk = smallp.tile([1, S], F32)
            nc.sync.dma_start(out=mk, in_=madd[b:b + 1, :])
            scm = sm1p.tile([1, S], F32)
            nc.vector.tensor_tensor(
                out=scm, in0=score_sb, in1=mk, op=mybir.AluOpType.add)
            ex = sm1p.tile([1, S], F32)
            ssum = smallp.tile([1, 1], F32)
            nc.scalar.activation(
                ex, scm, mybir.ActivationFunctionType.Exp,
                bias=0.0, scale=1.0, accum_out=ssum,
            )
            rinv = smallp.tile([1, 1], F32, tag="rinv", name=f"rinv{b}")
            nc.vector.reciprocal(rinv, ssum)
            attn_n = sm1p.tile([1, S], F32)
            nc.vector.tensor_scalar_mul(attn_n, ex, rinv)
            nc.sync.dma_start(out=attn_out[b:b + 1, :], in_=attn_n)
            return ex, rinv

        NHC = 4
        HC = H // NHC
        acols = {}

        def emit_ctx_hc(b, hc, ex_rinv):
            ex, rinv = ex_rinv
            # ---- context[b, h-chunk] = attn[b, :] @ enc[b][:, h-chunk] on PE
            # (attn columns as stationary; acol[p, j] = attn[b, p*16+j])
            if hc == 0:
                acols[b] = smallp.tile([128, S // 128], mm_dt, tag="acol",
                                       name=f"acol{b}")
                nc.sync.dma_start(out=acols[b], in_=ex.bitcast(mm_dt))
            acol = acols[b]
            h0 = hc * HC
            ectx = ectxp.tile([128, S // 128, HC], mm_dt)
            nc.sync.dma_start(
                out=ectx,
                in_=enc[b].rearrange("(p j) h -> p j h", p=128)
                [:, :, h0:h0 + HC].bitcast(mm_dt),
            )
            pcx = ppw.tile([1, HC], F32, tag="cxs")
            for j in range(S // 128):
                nc.tensor.matmul(
                    pcx,
                    lhsT=acol[:, j:j + 1],
                    rhs=ectx[:, j, :],
                    start=(j == 0),
                    stop=(j == S // 128 - 1),
                )
            cres = ctxp.tile([1, HC], F32)
            nc.vector.tensor_scalar_mul(cres, pcx, rinv)
            nc.sync.dma_start(out=ctx_out[b:b + 1, h0:h0 + HC], in_=cres)

        encT_tiles = {}

        def load_encT(b, rc, interleave_w1t=False):
            s0 = rc * RCHUNK
            # encT[hp, ht, r] = enc[b, s0+r, ht*128+hp], streamed from the
            # host-pretransposed copy (HWDGE; bitcast relabels f32 -> f32r)
            t = encTp.tile([128, HT, RCHUNK], mm_dt, tag="encT",
                           name=f"encT{b}_{rc}")
            for ht in range(HT):
                nc.sync.dma_start(
                    out=t[:, ht, :],
                    in_=encTd[b, ht * 128:(ht + 1) * 128,
                              s0:s0 + RCHUNK].bitcast(t.dtype),
                )
                if interleave_w1t and ht >= 1:
                    nc.sync.dma_start(
                        out=w1t_sb[:, ht, :],
                        in_=w1t[ht * 128:(ht + 1) * 128, :].bitcast(
                            w1t_sb.dtype))
            encT_tiles[(b, rc)] = t

        load_encT(0, 0, interleave_w1t=True)
        exs = []
        for b in range(bpc):
            score_sb = scorep.tile([1, S], F32)
            for rc in range(RC):
                s0 = rc * RCHUNK
                if (b, rc) not in encT_tiles:
                    load_encT(b, rc)
                nb, nrc = (b, rc + 1) if rc + 1 < RC else (b + 1, 0)
                if nb < bpc and not (b == 0 and rc == 0):
                    load_encT(nb, nrc)
                encT = encT_tiles.pop((b, rc))
                # pre^T = W1^T.T @ encT ; tanh(+W2h) ; score += v^T @ tanh
                pv = ppv.tile([1, RCHUNK], F32)
                for ot in range(OT):
                    pp = ppp.tile([128, RCHUNK], F32)
                    for ht in range(HT):
                        nc.tensor.matmul(
                            pp,
                            lhsT=w1t_sb[:, ht, ot * 128:(ot + 1) * 128],
                            rhs=encT[:, ht, :],
                            start=(ht == 0),
                            stop=(ht == HT - 1),
                        )
                    if b == 0 and rc == 0 and ot == 0:
                        emit_w2h()
                    tt = ttp.tile([128, RCHUNK], mm_dt)
                    nc.scalar.activation(
                        tt, pp, mybir.ActivationFunctionType.Tanh,
                        bias=w2h_sb[:, ot, b:b + 1], scale=1.0,
                    )
                    nc.tensor.matmul(
                        pv,
                        lhsT=vr_sb[:, ot:ot + 1],
                        rhs=tt,
                        start=(ot == 0),
                        stop=(ot == OT - 1),
                    )
                nc.vector.tensor_copy(score_sb[:, s0:s0 + RCHUNK], pv)
                if b > 0:
                    emit_ctx_hc(b - 1, rc, exs[b - 1])
            exs.append(emit_softmax(b, score_sb))
        for hc in range(NHC):
            emit_ctx_hc(bpc - 1, hc, exs[bpc - 1])

    nc.compile()
    return nc


_NC_CACHE = {}


def _get_nc(**kw):
    key = tuple(sorted(kw.items()))
    if key not in _NC_CACHE:
        _NC_CACHE[key] = build_program(**kw)
    return _NC_CACHE[key]


def make_in_maps(hidden_top, encoder_outputs, mask, W1, W2, v, bpc=BPC):
    """Host-side sharding + layout transforms (no model math)."""
    hidden_top = np.ascontiguousarray(hidden_top, dtype=np.float32)
    encoder_outputs = np.ascontiguousarray(encoder_outputs, dtype=np.float32)
    w1tf = np.ascontiguousarray(np.asarray(W1, dtype=np.float32).T)
    w2tf = np.ascontiguousarray(np.asarray(W2, dtype=np.float32).T)
    vrf = np.ascontiguousarray(np.asarray(v, dtype=np.float32).reshape(OT, 128).T)
    madd_f = np.where(np.asarray(mask), np.float32(0), np.float32(-1e9))
    n_cores = hidden_top.shape[0] // bpc
    in_maps = []
    encT_all = np.ascontiguousarray(encoder_outputs.transpose(0, 2, 1))
    for c in range(n_cores):
        sl = slice(c * bpc, (c + 1) * bpc)
        in_maps.append({
            "enc": encoder_outputs[sl],
            "encT": encT_all[sl],
            "hidT": np.ascontiguousarray(hidden_top[sl].T),
            "madd": madd_f[sl],
            "w1t": w1tf,
            "w2t": w2tf,
            "vr": vrf,
        })
    return in_maps


def kernel(hidden_top, encoder_outputs, mask, W1, W2, v):
    from concourse.bass_utils import run_bass_kernel_spmd

    nc = _get_nc()
    in_maps = make_in_maps(hidden_top, encoder_outputs, mask, W1, W2, v)
    res = run_bass_kernel_spmd(nc, in_maps, core_ids=list(range(NCORES)))
    context = np.concatenate([r["ctx_out"] for r in res.results], axis=0)
    attn = np.concatenate([r["attn_out"] for r in res.results], axis=0)
    return (context, attn)
